# revision 13
# baseline (speedup 1.0000x reference)
"""GQA attention (SEQ=2048, DIM=4096, 32 Q heads / 8 KV heads, head_dim=128),
tensor-parallel over heads across 8 NeuronCores.

Each core owns 4 Q heads + 1 KV head: wq/wk/wv split column-wise, wo split
row-wise; each core produces a partial (2048, 4096) output that the host sums
(the all-reduce of row-parallel wo).

Per-core kernel (matmuls on the float32r PE path unless noted):
  A) QKV projections: stream xT (dim-major) blocks; Q^T/K^T/V^T accumulate in
     PSUM over the 4096 contraction; RoPE applied on PSUM eviction; V^T
     transposed back to V-natural (bf16) with a ones column appended.
  B) Attention per (query-block, head): S^T = K^T_blk.T @ Q^T (keys on
     partitions), causal triangle mask on the diagonal 128-blocks only
     (diagonal score matmuls stream just the surviving columns), exp on ACT
     (scale=1/sqrt(128) folded in) emitting bf16 probs.  AV runs in natural-O
     orientation: lhsT = expS^T 128-col slice (bf16 -> fast weight load),
     rhs = [V_blk | 1] (bf16, 129 cols) so the softmax denominator accumulates
     for free in column 128 of PSUM.  Eviction normalizes with a per-partition
     ACT scale (1/D) and PE-transposes O back to O^T for phase C; transposes
     are deferred into the next head's stream to avoid head-of-line PE stalls.
  C) out = O^T.T @ wo accumulated over the 4 heads, streamed to DRAM.
"""

import numpy as np
import ml_dtypes

import concourse.bacc as bacc
import concourse.tile as tile
from concourse import mybir
from concourse.bass_utils import run_bass_kernel_spmd

F32 = mybir.dt.float32
F32R = mybir.dt.float32r
BF16 = mybir.dt.bfloat16

DIM = 4096
SEQ = 2048
HEAD_DIM = 128
N_CORES = 8
QH = 4              # q heads per core
QS = QH * HEAD_DIM  # 512: wq column slice per core
NKT = DIM // 128    # 32 contraction tiles
NSB = SEQ // 512    # 4 sequence blocks
NKB = SEQ // 128    # 16 key blocks
SCALE = 1.0 / float(np.sqrt(HEAD_DIM))
NEG = -1e9
LAG = 3             # AV matmuls trail the score stream by LAG key blocks


def build_nc():
    nc = bacc.Bacc(trn_type="TRN2")

    xT = nc.declare_dram_parameter("xT", [DIM, SEQ], BF16, isOutput=False)
    wq = nc.declare_dram_parameter("wq", [DIM, QS], BF16, isOutput=False)
    wk = nc.declare_dram_parameter("wk", [DIM, HEAD_DIM], BF16, isOutput=False)
    wv = nc.declare_dram_parameter("wv", [DIM, HEAD_DIM], BF16, isOutput=False)
    wo = nc.declare_dram_parameter("wo", [QS, DIM], F32R, isOutput=False)
    cosT = nc.declare_dram_parameter("cosT", [HEAD_DIM, SEQ], F32, isOutput=False)
    sinTs = nc.declare_dram_parameter("sinTs", [HEAD_DIM, SEQ], F32, isOutput=False)
    tri = nc.declare_dram_parameter("tri", [128, 128], F32, isOutput=False)
    ident = nc.declare_dram_parameter("ident", [128, 128], F32R, isOutput=False)
    onesv = nc.declare_dram_parameter("onesv", [128, NKB, 1], BF16, isOutput=False)
    out = nc.declare_dram_parameter("out", [SEQ, DIM], F32, isOutput=True)

    with tile.TileContext(nc) as tc:
        with (
            tc.tile_pool(name="persist", bufs=1) as persist,
            tc.tile_pool(name="resid", bufs=1) as resid,
            tc.tile_pool(name="vtbo", bufs=2) as vtbo,
        ):
            # resident activations (per-seq-block tiles so cross-phase
            # dependencies stay precise)
            qTs = [resid.tile([128, QH, 512], BF16, name=f"qT{sb}")
                   for sb in range(NSB)]
            kTs = [resid.tile([128, 512], BF16, name=f"kT{sb}")
                   for sb in range(NSB)]
            # V natural (keys, d) in bf16 with a ones column at 128
            vext = resid.tile([128, NKB, 130], BF16)

            tri_sb = persist.tile([128, 128], F32)
            ident_sb = persist.tile([128, 128], F32R)

            # ---------------- Phase A: projections + RoPE ----------------
            with (
                tc.tile_pool(name="wpool", bufs=1) as wpool,
                tc.tile_pool(name="xpool", bufs=4) as xpool,
                tc.tile_pool(name="cspool", bufs=2) as cspool,
                tc.tile_pool(name="ropetmp", bufs=2) as ropetmp,
                tc.tile_pool(name="psA", bufs=1, space="PSUM") as psA,
                tc.tile_pool(name="psVT", bufs=2, space="PSUM") as psVT,
            ):
                wq_r = wq.rearrange("(t p) m -> p t m", p=128)
                wk_r = wk.rearrange("(t p) m -> p t m", p=128)
                wv_r = wv.rearrange("(t p) m -> p t m", p=128)
                xT_r = xT.rearrange("(t p) s -> p t s", p=128)

                # g=0 weights/x as per-ktile tiles so the very first matmul
                # only waits on ~0.5 MB of DMA
                wq0 = [wpool.tile([128, QS], BF16, name=f"wq0_{i}")
                       for i in range(4)]
                xt0 = [wpool.tile([128, 512], BF16, name=f"xt0_{i}")
                       for i in range(4)]
                wq_cs, wk_cs, wv_cs = [None], [], []
                for c in range(8):
                    if c > 0:
                        wq_cs.append(wpool.tile([128, 4, QS], BF16, name=f"wqc{c}"))
                    wk_cs.append(wpool.tile([128, 4, HEAD_DIM], BF16, name=f"wkc{c}"))
                    wv_cs.append(wpool.tile([128, 4, HEAD_DIM], BF16, name=f"wvc{c}"))

                pending_vt = []  # deferred V transposes (per seq block)

                def flush_vt():
                    for fn in pending_vt:
                        fn()
                    pending_vt.clear()

                for sb in range(NSB):
                    ss = slice(sb * 512, (sb + 1) * 512)
                    q_ps = [psA.tile([128, 512], F32, tag=f"qps{h}", name=f"qps{h}")
                            for h in range(QH)]
                    k_ps = psA.tile([128, 512], F32, tag="kps")
                    v_ps = psA.tile([128, 512], F32, tag="vps")

                    for g in range(8):
                        if sb == 0:
                            if g == 0:
                                # fine-grained startup: wq/x per k-tile,
                                # smallest-first so matmul 0 starts early
                                nc.sync.dma_start(out=wq0[0], in_=wq_r[:, 0, :])
                                nc.scalar.dma_start(out=xt0[0], in_=xT_r[:, 0, ss])
                                nc.gpsimd.dma_start(out=wk_cs[0], in_=wk_r[:, 0:4, :])
                                nc.gpsimd.dma_start(out=wv_cs[0], in_=wv_r[:, 0:4, :])
                                for i in range(1, 4):
                                    nc.sync.dma_start(out=wq0[i], in_=wq_r[:, i, :])
                                    nc.scalar.dma_start(out=xt0[i], in_=xT_r[:, i, ss])
                            else:
                                nc.sync.dma_start(
                                    out=wq_cs[g], in_=wq_r[:, g * 4:(g + 1) * 4, :]
                                )
                                nc.gpsimd.dma_start(
                                    out=wk_cs[g], in_=wk_r[:, g * 4:(g + 1) * 4, :]
                                )
                                nc.gpsimd.dma_start(
                                    out=wv_cs[g], in_=wv_r[:, g * 4:(g + 1) * 4, :]
                                )
                            if g == 1:
                                # small constants: off the startup critical path
                                nc.sync.dma_start(out=tri_sb, in_=tri[:, :])
                                nc.sync.dma_start(out=ident_sb, in_=ident[:, :])
                                nc.sync.dma_start(
                                    out=vext[:, :, 128:129], in_=onesv[:, :, :]
                                )
                        if not (sb == 0 and g == 0):
                            xt = xpool.tile([128, 4, 512], BF16, tag="xt")
                            nc.scalar.dma_start(
                                out=xt, in_=xT_r[:, g * 4:(g + 1) * 4, ss]
                            )
                        if g == 2:
                            flush_vt()  # previous block's V transposes

                        def wq_ap(i, h):
                            if g == 0:
                                return wq0[i][:, h * 128:(h + 1) * 128]
                            return wq_cs[g][:, i, h * 128:(h + 1) * 128]

                        def x_ap(i):
                            if sb == 0 and g == 0:
                                return xt0[i]
                            return xt[:, i, :]

                        if g < 7:
                            for i in range(4):
                                kt = g * 4 + i
                                st = (kt == 0)
                                for h in range(QH):
                                    nc.tensor.matmul(
                                        q_ps[h], wq_ap(i, h), x_ap(i),
                                        start=st, stop=False,
                                    )
                                nc.tensor.matmul(
                                    k_ps, wk_cs[g][:, i, :], x_ap(i),
                                    start=st, stop=False,
                                )
                                nc.tensor.matmul(
                                    v_ps, wv_cs[g][:, i, :], x_ap(i),
                                    start=st, stop=False,
                                )
                        else:
                            # last super-tile: head-outer so q_ps[0] stops
                            # ~20 matmuls early and its RoPE overlaps the tail
                            for h in range(QH):
                                for i in range(4):
                                    nc.tensor.matmul(
                                        q_ps[h], wq_ap(i, h), x_ap(i),
                                        start=False, stop=(i == 3),
                                    )
                            for i in range(4):
                                nc.tensor.matmul(
                                    k_ps, wk_cs[g][:, i, :], x_ap(i),
                                    start=False, stop=(i == 3),
                                )
                            for i in range(4):
                                nc.tensor.matmul(
                                    v_ps, wv_cs[g][:, i, :], x_ap(i),
                                    start=False, stop=(i == 3),
                                )

                    # RoPE tables for this block
                    cos_t = cspool.tile([128, 512], F32, tag="cos")
                    nc.sync.dma_start(out=cos_t, in_=cosT[:, ss])
                    sin_t = cspool.tile([128, 512], F32, tag="sin")
                    nc.sync.dma_start(out=sin_t, in_=sinTs[:, ss])

                    def rope(dst, src_ps):
                        # DVE multiplies the straight view directly from PSUM
                        # (partition-aligned); only the half-rotated view needs
                        # ACT cross-partition copies.  PSUM bank frees after
                        # max(ACT copies, DVE mul).
                        vr = ropetmp.tile([128, 512], F32, tag="vr", name="vr")
                        nc.scalar.copy(vr[0:64, :], src_ps[64:128, :])
                        nc.scalar.copy(vr[64:128, :], src_ps[0:64, :])
                        t = ropetmp.tile([128, 512], F32, tag="t", name="t")
                        u = ropetmp.tile([128, 512], F32, tag="u", name="u")
                        nc.vector.tensor_mul(t, src_ps, cos_t)
                        nc.vector.tensor_mul(u, vr, sin_t)
                        nc.vector.tensor_add(dst, t, u)

                    for h in range(QH):
                        rope(qTs[sb][:, h, :], q_ps[h])
                    rope(kTs[sb], k_ps)

                    # V^T -> V-natural bf16 via PE transposes; deferred into
                    # the next block's matmul stream (or phase B for sb=3)
                    vt_sb = vtbo.tile([128, 512], F32R, tag="vt")
                    nc.scalar.copy(vt_sb, v_ps)

                    def mk_vt(sb, vt_sb):
                        def emit(pool=psVT, tag="vtp"):
                            for j in range(4):
                                vt_ps = pool.tile([128, 128], F32R,
                                                  tag=tag, name=tag)
                                nc.tensor.transpose(
                                    vt_ps, vt_sb[:, j * 128:(j + 1) * 128],
                                    ident_sb,
                                )
                                nc.scalar.copy(
                                    vext[:, sb * 4 + j, 0:128], vt_ps
                                )
                        return emit

                    if sb < NSB - 1:
                        pending_vt.append(mk_vt(sb, vt_sb))
                    else:
                        last_vt = mk_vt(sb, vt_sb)

            # ---------------- Phase B/C: attention + out projection ----------------
            with (
                tc.tile_pool(name="wopool", bufs=1) as wopool,
                tc.tile_pool(name="expp", bufs=8) as expp,
                tc.tile_pool(name="dpool", bufs=6) as dpool,
                tc.tile_pool(name="ospool", bufs=6) as ospool,
                tc.tile_pool(name="otpool", bufs=2) as otpool,
                tc.tile_pool(name="outev", bufs=4) as outev,
                tc.tile_pool(name="psS", bufs=2, space="PSUM") as psS,
                tc.tile_pool(name="psO", bufs=1, space="PSUM") as psO,
                tc.tile_pool(name="psT", bufs=2, space="PSUM") as psT,
            ):
                wo_sb = wopool.tile([128, QH, DIM], F32R)
                wo_r = wo.rearrange("(h p) n -> p h n", p=128)
                for h in range(QH):
                    for c in range(2):
                        nc.sync.dma_start(
                            out=wo_sb[:, h, c * 2048:(c + 1) * 2048],
                            in_=wo_r[:, h, c * 2048:(c + 1) * 2048],
                        )

                first_S = [True]
                pending_t = []  # deferred O transposes

                def flush_t():
                    for fn in pending_t:
                        fn()
                    pending_t.clear()

                for qb in reversed(range(NSB)):
                    n_kb = 4 * qb + 4
                    otT_sb = otpool.tile([128, QH, 512], F32R, tag="ott",
                                         name="ott")
                    for h in range(QH):
                        o_acc = [psO.tile([128, 129], F32, tag=f"oacc{qc}",
                                          name=f"oacc{qc}")
                                 for qc in range(QH)]
                        ess = [None] * n_kb

                        def drain(kb, h=h, qb=qb, o_acc=o_acc, ess=ess,
                                  otT_sb=otT_sb):
                            for qc in range(4):
                                if 4 * qb + qc < kb:
                                    continue
                                stop = (kb == 4 * qb + qc)
                                nc.tensor.matmul(
                                    o_acc[qc],
                                    ess[kb][:, qc * 128:(qc + 1) * 128],
                                    vext[:, kb, 0:129],
                                    start=(kb == 0), stop=stop,
                                )
                                if stop:
                                    dinv = dpool.tile([128, 1], F32,
                                                      tag="dinv", name="dinv")
                                    nc.vector.reciprocal(
                                        dinv, o_acc[qc][:, 128:129]
                                    )
                                    o_sb = ospool.tile([128, 128], F32R,
                                                       tag="osb", name="osb")
                                    nc.scalar.activation(
                                        o_sb, o_acc[qc][:, 0:128],
                                        mybir.ActivationFunctionType.Copy,
                                        scale=dinv,
                                    )

                                    def mk_tp(h=h, qc=qc, o_sb=o_sb,
                                              otT_sb=otT_sb):
                                        def emit():
                                            tp = psT.tile([128, 128], F32R,
                                                          tag="tps", name="tps")
                                            nc.tensor.transpose(
                                                tp, o_sb, ident_sb
                                            )
                                            nc.vector.tensor_copy(
                                                otT_sb[:, h,
                                                       qc * 128:(qc + 1) * 128],
                                                tp,
                                            )
                                        return emit

                                    pending_t.append(mk_tp())

                        for kb in range(n_kb):
                            j = kb - 4 * qb
                            lo = max(0, j) * 128
                            s_ps = psS.tile([128, 512], F32, tag="sps",
                                            name="sps")
                            nc.tensor.matmul(
                                s_ps[:, lo:512],
                                kTs[kb // 4][:, (kb % 4) * 128:
                                             (kb % 4 + 1) * 128],
                                qTs[qb][:, h, lo:512],
                                start=True, stop=True,
                            )
                            if j >= 0:
                                nc.vector.tensor_add(
                                    s_ps[:, lo:lo + 128],
                                    s_ps[:, lo:lo + 128],
                                    tri_sb,
                                )
                            es = expp.tile([128, 512], BF16, tag="es",
                                           name="es")
                            nc.scalar.activation(
                                es[:, lo:512], s_ps[:, lo:512],
                                mybir.ActivationFunctionType.Exp,
                                scale=SCALE,
                            )
                            ess[kb] = es
                            if kb == 3:
                                flush_t()  # previous head's O transposes
                                if first_S[0]:
                                    # sb=3 V transposes: deps long-ready by
                                    # now, slot in without stalling the PE
                                    last_vt(psT, "tps")
                                    first_S[0] = False
                            if kb >= LAG:
                                drain(kb - LAG)
                        for kb in range(max(0, n_kb - LAG), n_kb):
                            drain(kb)

                    flush_t()

                    # Phase C for this query block
                    for qc in range(4):
                        for nb in range(8):
                            o_ps = psO.tile([128, 512], F32,
                                            tag=f"oacc{(qc * 8 + nb) % 4}",
                                            name="ops")
                            for h in range(QH):
                                nc.tensor.matmul(
                                    o_ps,
                                    otT_sb[:, h, qc * 128:(qc + 1) * 128],
                                    wo_sb[:, h, nb * 512:(nb + 1) * 512],
                                    start=(h == 0), stop=(h == QH - 1),
                                )
                            ob = outev.tile([128, 512], F32, tag="ob",
                                            name="ob")
                            if nb % 2 == 0:
                                nc.vector.tensor_copy(ob, o_ps)
                            else:
                                nc.scalar.copy(ob, o_ps)
                            dma_eng = nc.sync if nb % 2 == 0 else nc.gpsimd
                            dma_eng.dma_start(
                                out=out[qb * 512 + qc * 128:
                                        qb * 512 + (qc + 1) * 128,
                                        nb * 512:(nb + 1) * 512],
                                in_=ob,
                            )
    nc.finalize()
    return nc


_NC_CACHE = {}


def _get_nc():
    if "nc" not in _NC_CACHE:
        _NC_CACHE["nc"] = build_nc()
    return _NC_CACHE["nc"]


def _host_prep(x, cos, sin, mask, wq, wk, wv, wo):
    xT = np.ascontiguousarray(x[0].T.astype(ml_dtypes.bfloat16))
    cosT = np.ascontiguousarray(cos[:, 0, :].T.astype(np.float32))
    sinT = sin[:, 0, :].T.astype(np.float32)
    sinTs = np.ascontiguousarray(
        np.concatenate([-sinT[:64], sinT[64:]], axis=0)
    )
    rr = np.arange(128, dtype=np.int64)[:, None]
    cc = np.arange(128, dtype=np.int64)[None, :]
    tri = np.where(rr > cc, NEG, 0.0).astype(np.float32)
    ident = np.eye(128, dtype=np.float32)
    onesv = np.ones((128, NKB, 1), dtype=ml_dtypes.bfloat16)

    in_maps = []
    for i in range(N_CORES):
        in_maps.append({
            "xT": xT,
            "wq": np.ascontiguousarray(wq[:, i * QS:(i + 1) * QS].astype(ml_dtypes.bfloat16)),
            "wk": np.ascontiguousarray(wk[:, i * 128:(i + 1) * 128].astype(ml_dtypes.bfloat16)),
            "wv": np.ascontiguousarray(wv[:, i * 128:(i + 1) * 128].astype(ml_dtypes.bfloat16)),
            "wo": np.ascontiguousarray(wo[i * QS:(i + 1) * QS, :]),
            "cosT": cosT,
            "sinTs": sinTs,
            "tri": tri,
            "ident": ident,
            "onesv": onesv,
        })
    return in_maps


def kernel(x, cos, sin, mask, wq, wk, wv, wo, _trace=False, _trace_kwargs=None):
    nc = _get_nc()
    in_maps = _host_prep(x, cos, sin, mask, wq, wk, wv, wo)
    res = run_bass_kernel_spmd(
        nc, in_maps, list(range(N_CORES)), trace=_trace,
        **(_trace_kwargs or {}),
    )
    partials = [res.results[i]["out"] for i in range(N_CORES)]
    full = np.sum(np.stack(partials, axis=0), axis=0, dtype=np.float64)
    out = full.astype(np.float32)[None, :, :]
    if _trace:
        return out, res
    return out


# revision 15
# speedup vs baseline: 1.0221x; 1.0221x over previous
"""GQA attention (SEQ=2048, DIM=4096, 32 Q heads / 8 KV heads, head_dim=128),
tensor-parallel over heads across 8 NeuronCores.

Each core owns 4 Q heads + 1 KV head: wq/wk/wv split column-wise, wo split
row-wise; each core produces a partial (2048, 4096) output that the host sums
(the all-reduce of row-parallel wo).

Per-core kernel (matmuls on the float32r PE path unless noted):
  A) QKV projections: stream xT (dim-major) blocks; Q^T/K^T/V^T accumulate in
     PSUM over the 4096 contraction; RoPE applied on PSUM eviction; V^T
     transposed back to V-natural (bf16) with a ones column appended.
  B) Attention per (query-block, head): S^T = K^T_blk.T @ Q^T (keys on
     partitions), causal triangle mask on the diagonal 128-blocks only
     (diagonal score matmuls stream just the surviving columns), exp on ACT
     (scale=1/sqrt(128) folded in) emitting bf16 probs.  AV runs in natural-O
     orientation: lhsT = expS^T 128-col slice (bf16 -> fast weight load),
     rhs = [V_blk | 1] (bf16, 129 cols) so the softmax denominator accumulates
     for free in column 128 of PSUM.  Eviction normalizes with a per-partition
     ACT scale (1/D) and PE-transposes O back to O^T for phase C; transposes
     are deferred into the next head's stream to avoid head-of-line PE stalls.
  C) out = O^T.T @ wo accumulated over the 4 heads, streamed to DRAM.
"""

import numpy as np
import ml_dtypes

import concourse.bacc as bacc
import concourse.tile as tile
from concourse import mybir
from concourse.bass_utils import run_bass_kernel_spmd

F32 = mybir.dt.float32
F32R = mybir.dt.float32r
BF16 = mybir.dt.bfloat16

DIM = 4096
SEQ = 2048
HEAD_DIM = 128
N_CORES = 8
QH = 4              # q heads per core
QS = QH * HEAD_DIM  # 512: wq column slice per core
NKT = DIM // 128    # 32 contraction tiles
NSB = SEQ // 512    # 4 sequence blocks
NKB = SEQ // 128    # 16 key blocks
SCALE = 1.0 / float(np.sqrt(HEAD_DIM))
NEG = -1e9
LAG = 3             # AV matmuls trail the score stream by LAG key blocks


def build_nc():
    nc = bacc.Bacc(trn_type="TRN2")

    xT = nc.declare_dram_parameter("xT", [DIM, SEQ], BF16, isOutput=False)
    wq = nc.declare_dram_parameter("wq", [DIM, QS], BF16, isOutput=False)
    wk = nc.declare_dram_parameter("wk", [DIM, HEAD_DIM], BF16, isOutput=False)
    wv = nc.declare_dram_parameter("wv", [DIM, HEAD_DIM], BF16, isOutput=False)
    wo = nc.declare_dram_parameter("wo", [QS, DIM], F32R, isOutput=False)
    cosT = nc.declare_dram_parameter("cosT", [HEAD_DIM, SEQ], F32, isOutput=False)
    sinTs = nc.declare_dram_parameter("sinTs", [HEAD_DIM, SEQ], F32, isOutput=False)
    tri = nc.declare_dram_parameter("tri", [128, 128], F32, isOutput=False)
    ident = nc.declare_dram_parameter("ident", [128, 128], F32R, isOutput=False)
    onesv = nc.declare_dram_parameter("onesv", [128, NKB, 1], BF16, isOutput=False)
    out = nc.declare_dram_parameter("out", [SEQ, DIM], F32, isOutput=True)

    with tile.TileContext(nc) as tc:
        with (
            tc.tile_pool(name="persist", bufs=1) as persist,
            tc.tile_pool(name="resid", bufs=1) as resid,
            tc.tile_pool(name="vtbo", bufs=2) as vtbo,
        ):
            # resident activations (per-seq-block tiles so cross-phase
            # dependencies stay precise)
            qTs = [resid.tile([128, QH, 512], BF16, name=f"qT{sb}")
                   for sb in range(NSB)]
            kTs = [resid.tile([128, 512], BF16, name=f"kT{sb}")
                   for sb in range(NSB)]
            # V natural (keys, d) in bf16 with a ones column at 128
            vext = resid.tile([128, NKB, 130], BF16)

            tri_sb = persist.tile([128, 128], F32)
            ident_sb = persist.tile([128, 128], F32R)

            # ---------------- Phase A: projections + RoPE ----------------
            with (
                tc.tile_pool(name="wpool", bufs=1) as wpool,
                tc.tile_pool(name="xpool", bufs=4) as xpool,
                tc.tile_pool(name="cspool", bufs=2) as cspool,
                tc.tile_pool(name="ropetmp", bufs=2) as ropetmp,
                tc.tile_pool(name="psA", bufs=1, space="PSUM") as psA,
                tc.tile_pool(name="psVT", bufs=2, space="PSUM") as psVT,
            ):
                wq_r = wq.rearrange("(t p) m -> p t m", p=128)
                wk_r = wk.rearrange("(t p) m -> p t m", p=128)
                wv_r = wv.rearrange("(t p) m -> p t m", p=128)
                xT_r = xT.rearrange("(t p) s -> p t s", p=128)

                # g=0 weights/x as per-ktile tiles so the very first matmul
                # only waits on ~0.5 MB of DMA
                wq0 = [wpool.tile([128, QS], BF16, name=f"wq0_{i}")
                       for i in range(4)]
                xt0 = [wpool.tile([128, 512], BF16, name=f"xt0_{i}")
                       for i in range(4)]
                wq_cs, wk_cs, wv_cs = [None], [], []
                for c in range(8):
                    if c > 0:
                        wq_cs.append(wpool.tile([128, 4, QS], BF16, name=f"wqc{c}"))
                    wk_cs.append(wpool.tile([128, 4, HEAD_DIM], BF16, name=f"wkc{c}"))
                    wv_cs.append(wpool.tile([128, 4, HEAD_DIM], BF16, name=f"wvc{c}"))

                pending_vt = []  # deferred V transposes (per seq block)

                def flush_vt():
                    for fn in pending_vt:
                        fn()
                    pending_vt.clear()

                for sb in range(NSB):
                    ss = slice(sb * 512, (sb + 1) * 512)
                    q_ps = [psA.tile([128, 512], F32, tag=f"qps{h}", name=f"qps{h}")
                            for h in range(QH)]
                    k_ps = psA.tile([128, 512], F32, tag="kps")
                    v_ps = psA.tile([128, 512], F32, tag="vps")

                    for g in range(8):
                        if sb == 0:
                            if g == 0:
                                # fine-grained startup: wq/x per k-tile,
                                # smallest-first so matmul 0 starts early
                                nc.sync.dma_start(out=wq0[0], in_=wq_r[:, 0, :])
                                nc.scalar.dma_start(out=xt0[0], in_=xT_r[:, 0, ss])
                                nc.sync.dma_start(out=wk_cs[0], in_=wk_r[:, 0:4, :])
                                nc.sync.dma_start(out=wv_cs[0], in_=wv_r[:, 0:4, :])
                                for i in range(1, 4):
                                    nc.sync.dma_start(out=wq0[i], in_=wq_r[:, i, :])
                                    nc.scalar.dma_start(out=xt0[i], in_=xT_r[:, i, ss])
                            else:
                                nc.sync.dma_start(
                                    out=wq_cs[g], in_=wq_r[:, g * 4:(g + 1) * 4, :]
                                )
                                nc.sync.dma_start(
                                    out=wk_cs[g], in_=wk_r[:, g * 4:(g + 1) * 4, :]
                                )
                                nc.sync.dma_start(
                                    out=wv_cs[g], in_=wv_r[:, g * 4:(g + 1) * 4, :]
                                )
                            if g == 1:
                                # small constants: off the startup critical path
                                nc.sync.dma_start(out=tri_sb, in_=tri[:, :])
                                nc.sync.dma_start(out=ident_sb, in_=ident[:, :])
                                nc.sync.dma_start(
                                    out=vext[:, :, 128:129], in_=onesv[:, :, :]
                                )
                        if not (sb == 0 and g == 0):
                            xt = xpool.tile([128, 4, 512], BF16, tag="xt")
                            nc.scalar.dma_start(
                                out=xt, in_=xT_r[:, g * 4:(g + 1) * 4, ss]
                            )
                        if g == 2:
                            flush_vt()  # previous block's V transposes

                        def wq_ap(i, h):
                            if g == 0:
                                return wq0[i][:, h * 128:(h + 1) * 128]
                            return wq_cs[g][:, i, h * 128:(h + 1) * 128]

                        def x_ap(i):
                            if sb == 0 and g == 0:
                                return xt0[i]
                            return xt[:, i, :]

                        if g < 7:
                            for i in range(4):
                                kt = g * 4 + i
                                st = (kt == 0)
                                for h in range(QH):
                                    nc.tensor.matmul(
                                        q_ps[h], wq_ap(i, h), x_ap(i),
                                        start=st, stop=False,
                                    )
                                nc.tensor.matmul(
                                    k_ps, wk_cs[g][:, i, :], x_ap(i),
                                    start=st, stop=False,
                                )
                                nc.tensor.matmul(
                                    v_ps, wv_cs[g][:, i, :], x_ap(i),
                                    start=st, stop=False,
                                )
                        else:
                            # last super-tile: head-outer so q_ps[0] stops
                            # ~20 matmuls early and its RoPE overlaps the tail
                            for h in range(QH):
                                for i in range(4):
                                    nc.tensor.matmul(
                                        q_ps[h], wq_ap(i, h), x_ap(i),
                                        start=False, stop=(i == 3),
                                    )
                            for i in range(4):
                                nc.tensor.matmul(
                                    k_ps, wk_cs[g][:, i, :], x_ap(i),
                                    start=False, stop=(i == 3),
                                )
                            for i in range(4):
                                nc.tensor.matmul(
                                    v_ps, wv_cs[g][:, i, :], x_ap(i),
                                    start=False, stop=(i == 3),
                                )

                    # RoPE tables for this block
                    cos_t = cspool.tile([128, 512], F32, tag="cos")
                    nc.sync.dma_start(out=cos_t, in_=cosT[:, ss])
                    sin_t = cspool.tile([128, 512], F32, tag="sin")
                    nc.sync.dma_start(out=sin_t, in_=sinTs[:, ss])

                    def rope(dst, src_ps):
                        # DVE multiplies the straight view directly from PSUM
                        # (partition-aligned); only the half-rotated view needs
                        # ACT cross-partition copies.  PSUM bank frees after
                        # max(ACT copies, DVE mul).
                        vr = ropetmp.tile([128, 512], F32, tag="vr", name="vr")
                        nc.scalar.copy(vr[0:64, :], src_ps[64:128, :])
                        nc.scalar.copy(vr[64:128, :], src_ps[0:64, :])
                        t = ropetmp.tile([128, 512], F32, tag="t", name="t")
                        u = ropetmp.tile([128, 512], F32, tag="u", name="u")
                        nc.vector.tensor_mul(t, src_ps, cos_t)
                        nc.vector.tensor_mul(u, vr, sin_t)
                        nc.vector.tensor_add(dst, t, u)

                    for h in range(QH):
                        rope(qTs[sb][:, h, :], q_ps[h])
                    rope(kTs[sb], k_ps)

                    # V^T -> V-natural bf16 via PE transposes; deferred into
                    # the next block's matmul stream (or phase B for sb=3)
                    vt_sb = vtbo.tile([128, 512], F32R, tag="vt")
                    nc.scalar.copy(vt_sb, v_ps)

                    def mk_vt(sb, vt_sb):
                        def emit(pool=psVT, tag="vtp"):
                            for j in range(4):
                                vt_ps = pool.tile([128, 128], F32R,
                                                  tag=tag, name=tag)
                                nc.tensor.transpose(
                                    vt_ps, vt_sb[:, j * 128:(j + 1) * 128],
                                    ident_sb,
                                )
                                nc.scalar.copy(
                                    vext[:, sb * 4 + j, 0:128], vt_ps
                                )
                        return emit

                    if sb < NSB - 1:
                        pending_vt.append(mk_vt(sb, vt_sb))
                    else:
                        last_vt = mk_vt(sb, vt_sb)
                        # keep the PE busy through the pool-boundary barrier
                        # (rope tail) so HAM stays at full clock into phase B
                        for i in range(20):
                            dm_ps = psVT.tile([128, 128], F32R, tag="vtp",
                                              name="vtp")
                            nc.tensor.transpose(
                                dm_ps, ident_sb, ident_sb
                            )

            # ---------------- Phase B/C: attention + out projection ----------------
            with (
                tc.tile_pool(name="wopool", bufs=1) as wopool,
                tc.tile_pool(name="expp", bufs=8) as expp,
                tc.tile_pool(name="dpool", bufs=6) as dpool,
                tc.tile_pool(name="ospool", bufs=6) as ospool,
                tc.tile_pool(name="otpool", bufs=2) as otpool,
                tc.tile_pool(name="outev", bufs=4) as outev,
                tc.tile_pool(name="psS", bufs=2, space="PSUM") as psS,
                tc.tile_pool(name="psO", bufs=1, space="PSUM") as psO,
                tc.tile_pool(name="psT", bufs=2, space="PSUM") as psT,
            ):
                wo_sb = wopool.tile([128, QH, DIM], F32R)
                wo_r = wo.rearrange("(h p) n -> p h n", p=128)
                for h in range(QH):
                    for c in range(2):
                        nc.sync.dma_start(
                            out=wo_sb[:, h, c * 2048:(c + 1) * 2048],
                            in_=wo_r[:, h, c * 2048:(c + 1) * 2048],
                        )

                first_S = [True]
                pending_t = []  # deferred O transposes

                def flush_t():
                    for fn in pending_t:
                        fn()
                    pending_t.clear()

                for qb in range(NSB):
                    n_kb = 4 * qb + 4
                    otT_sb = otpool.tile([128, QH, 512], F32R, tag="ott",
                                         name="ott")
                    for h in range(QH):
                        o_acc = [psO.tile([128, 129], F32, tag=f"oacc{qc}",
                                          name=f"oacc{qc}")
                                 for qc in range(QH)]
                        ess = [None] * n_kb

                        def drain(kb, h=h, qb=qb, o_acc=o_acc, ess=ess,
                                  otT_sb=otT_sb):
                            for qc in range(4):
                                if 4 * qb + qc < kb:
                                    continue
                                stop = (kb == 4 * qb + qc)
                                nc.tensor.matmul(
                                    o_acc[qc],
                                    ess[kb][:, qc * 128:(qc + 1) * 128],
                                    vext[:, kb, 0:129],
                                    start=(kb == 0), stop=stop,
                                )
                                if stop:
                                    dinv = dpool.tile([128, 1], F32,
                                                      tag="dinv", name="dinv")
                                    nc.vector.reciprocal(
                                        dinv, o_acc[qc][:, 128:129]
                                    )
                                    o_sb = ospool.tile([128, 128], F32R,
                                                       tag="osb", name="osb")
                                    nc.scalar.activation(
                                        o_sb, o_acc[qc][:, 0:128],
                                        mybir.ActivationFunctionType.Copy,
                                        scale=dinv,
                                    )

                                    def mk_tp(h=h, qc=qc, o_sb=o_sb,
                                              otT_sb=otT_sb):
                                        def emit():
                                            tp = psT.tile([128, 128], F32R,
                                                          tag="tps", name="tps")
                                            nc.tensor.transpose(
                                                tp, o_sb, ident_sb
                                            )
                                            nc.vector.tensor_copy(
                                                otT_sb[:, h,
                                                       qc * 128:(qc + 1) * 128],
                                                tp,
                                            )
                                        return emit

                                    pending_t.append(mk_tp())

                        for kb in range(n_kb):
                            j = kb - 4 * qb
                            lo = max(0, j) * 128
                            s_ps = psS.tile([128, 512], F32, tag="sps",
                                            name="sps")
                            nc.tensor.matmul(
                                s_ps[:, lo:512],
                                kTs[kb // 4][:, (kb % 4) * 128:
                                             (kb % 4 + 1) * 128],
                                qTs[qb][:, h, lo:512],
                                start=True, stop=True,
                            )
                            if j >= 0:
                                nc.vector.tensor_add(
                                    s_ps[:, lo:lo + 128],
                                    s_ps[:, lo:lo + 128],
                                    tri_sb,
                                )
                            es = expp.tile([128, 512], BF16, tag="es",
                                           name="es")
                            nc.scalar.activation(
                                es[:, lo:512], s_ps[:, lo:512],
                                mybir.ActivationFunctionType.Exp,
                                scale=SCALE,
                            )
                            ess[kb] = es
                            if kb == 3:
                                flush_t()  # previous head's O transposes
                                if first_S[0]:
                                    # sb=3 V transposes: deps long-ready by
                                    # now, slot in without stalling the PE
                                    last_vt(psT, "tps")
                                    first_S[0] = False
                            lag = 6 if h == 0 else LAG
                            if kb >= lag:
                                drain(kb - lag)
                        for kb in range(max(0, n_kb - lag), n_kb):
                            drain(kb)

                    flush_t()

                    # Phase C for this query block
                    for qc in range(4):
                        for nb in range(8):
                            o_ps = psO.tile([128, 512], F32,
                                            tag=f"oacc{(qc * 8 + nb) % 4}",
                                            name="ops")
                            for h in range(QH):
                                nc.tensor.matmul(
                                    o_ps,
                                    otT_sb[:, h, qc * 128:(qc + 1) * 128],
                                    wo_sb[:, h, nb * 512:(nb + 1) * 512],
                                    start=(h == 0), stop=(h == QH - 1),
                                )
                            ob = outev.tile([128, 512], F32, tag="ob",
                                            name="ob")
                            nc.vector.tensor_copy(ob, o_ps)
                            dma_eng = nc.sync if nb % 2 == 0 else nc.gpsimd
                            dma_eng.dma_start(
                                out=out[qb * 512 + qc * 128:
                                        qb * 512 + (qc + 1) * 128,
                                        nb * 512:(nb + 1) * 512],
                                in_=ob,
                            )
    nc.finalize()
    return nc


_NC_CACHE = {}


def _get_nc():
    if "nc" not in _NC_CACHE:
        _NC_CACHE["nc"] = build_nc()
    return _NC_CACHE["nc"]


def _host_prep(x, cos, sin, mask, wq, wk, wv, wo):
    xT = np.ascontiguousarray(x[0].T.astype(ml_dtypes.bfloat16))
    cosT = np.ascontiguousarray(cos[:, 0, :].T.astype(np.float32))
    sinT = sin[:, 0, :].T.astype(np.float32)
    sinTs = np.ascontiguousarray(
        np.concatenate([-sinT[:64], sinT[64:]], axis=0)
    )
    rr = np.arange(128, dtype=np.int64)[:, None]
    cc = np.arange(128, dtype=np.int64)[None, :]
    tri = np.where(rr > cc, NEG, 0.0).astype(np.float32)
    ident = np.eye(128, dtype=np.float32)
    onesv = np.ones((128, NKB, 1), dtype=ml_dtypes.bfloat16)

    in_maps = []
    for i in range(N_CORES):
        in_maps.append({
            "xT": xT,
            "wq": np.ascontiguousarray(wq[:, i * QS:(i + 1) * QS].astype(ml_dtypes.bfloat16)),
            "wk": np.ascontiguousarray(wk[:, i * 128:(i + 1) * 128].astype(ml_dtypes.bfloat16)),
            "wv": np.ascontiguousarray(wv[:, i * 128:(i + 1) * 128].astype(ml_dtypes.bfloat16)),
            "wo": np.ascontiguousarray(wo[i * QS:(i + 1) * QS, :]),
            "cosT": cosT,
            "sinTs": sinTs,
            "tri": tri,
            "ident": ident,
            "onesv": onesv,
        })
    return in_maps


def kernel(x, cos, sin, mask, wq, wk, wv, wo, _trace=False, _trace_kwargs=None):
    nc = _get_nc()
    in_maps = _host_prep(x, cos, sin, mask, wq, wk, wv, wo)
    res = run_bass_kernel_spmd(
        nc, in_maps, list(range(N_CORES)), trace=_trace,
        **(_trace_kwargs or {}),
    )
    partials = [res.results[i]["out"] for i in range(N_CORES)]
    full = np.sum(np.stack(partials, axis=0), axis=0, dtype=np.float64)
    out = full.astype(np.float32)[None, :, :]
    if _trace:
        return out, res
    return out


# revision 17
# speedup vs baseline: 1.0486x; 1.0259x over previous
"""GQA attention (SEQ=2048, DIM=4096, 32 Q heads / 8 KV heads, head_dim=128),
tensor-parallel over heads across 8 NeuronCores.

Each core owns 4 Q heads + 1 KV head: wq/wk/wv split column-wise, wo split
row-wise; each core produces a partial (2048, 4096) output that the host sums
(the all-reduce of row-parallel wo).

Per-core kernel (matmuls on the float32r PE path unless noted):
  A) QKV projections: stream xT (dim-major) blocks; Q^T/K^T/V^T accumulate in
     PSUM over the 4096 contraction; RoPE applied on PSUM eviction; V^T
     transposed back to V-natural (bf16) with a ones column appended.
  B) Attention per (query-block, head): S^T = K^T_blk.T @ Q^T (keys on
     partitions), causal triangle mask on the diagonal 128-blocks only
     (diagonal score matmuls stream just the surviving columns), exp on ACT
     (scale=1/sqrt(128) folded in) emitting bf16 probs.  AV runs in natural-O
     orientation: lhsT = expS^T 128-col slice (bf16 -> fast weight load),
     rhs = [V_blk | 1] (bf16, 129 cols) so the softmax denominator accumulates
     for free in column 128 of PSUM.  Eviction normalizes with a per-partition
     ACT scale (1/D) and PE-transposes O back to O^T for phase C; transposes
     are deferred into the next head's stream to avoid head-of-line PE stalls.
  C) out = O^T.T @ wo accumulated over the 4 heads, streamed to DRAM.
"""

import numpy as np
import ml_dtypes

import concourse.bacc as bacc
import concourse.tile as tile
from concourse import mybir
from concourse.bass_utils import run_bass_kernel_spmd

F32 = mybir.dt.float32
F32R = mybir.dt.float32r
BF16 = mybir.dt.bfloat16

DIM = 4096
SEQ = 2048
HEAD_DIM = 128
N_CORES = 8
QH = 4              # q heads per core
QS = QH * HEAD_DIM  # 512: wq column slice per core
NKT = DIM // 128    # 32 contraction tiles
NSB = SEQ // 512    # 4 sequence blocks
NKB = SEQ // 128    # 16 key blocks
SCALE = 1.0 / float(np.sqrt(HEAD_DIM))
NEG = -1e9
LAG = 3             # AV matmuls trail the score stream by LAG key blocks


def build_nc():
    nc = bacc.Bacc(trn_type="TRN2")

    xT = nc.declare_dram_parameter("xT", [DIM, SEQ], BF16, isOutput=False)
    wq = nc.declare_dram_parameter("wq", [DIM, QS], BF16, isOutput=False)
    wk = nc.declare_dram_parameter("wk", [DIM, HEAD_DIM], BF16, isOutput=False)
    wv = nc.declare_dram_parameter("wv", [DIM, HEAD_DIM], BF16, isOutput=False)
    wo = nc.declare_dram_parameter("wo", [QS, DIM], F32R, isOutput=False)
    cosT = nc.declare_dram_parameter("cosT", [HEAD_DIM, SEQ], F32, isOutput=False)
    sinTs = nc.declare_dram_parameter("sinTs", [HEAD_DIM, SEQ], F32, isOutput=False)
    tri = nc.declare_dram_parameter("tri", [128, 128], F32, isOutput=False)
    ident = nc.declare_dram_parameter("ident", [128, 128], F32R, isOutput=False)
    onesv = nc.declare_dram_parameter("onesv", [128, NKB, 1], BF16, isOutput=False)
    out = nc.declare_dram_parameter("out", [SEQ, DIM], F32, isOutput=True)

    with tile.TileContext(nc) as tc:
        with (
            tc.tile_pool(name="persist", bufs=1) as persist,
            tc.tile_pool(name="resid", bufs=1) as resid,
            tc.tile_pool(name="vtbo", bufs=2) as vtbo,
        ):
            # resident activations (per-seq-block tiles so cross-phase
            # dependencies stay precise)
            qTs = [resid.tile([128, QH, 512], BF16, name=f"qT{sb}")
                   for sb in range(NSB)]
            kTs = [resid.tile([128, 512], BF16, name=f"kT{sb}")
                   for sb in range(NSB)]
            # V natural (keys, d) in bf16 with a ones column at 128
            vext = resid.tile([128, NKB, 130], BF16)

            tri_sb = persist.tile([128, 128], F32)
            ident_sb = persist.tile([128, 128], F32R)

            # ---------------- Phase A: projections + RoPE ----------------
            with (
                tc.tile_pool(name="wpool", bufs=1) as wpool,
                tc.tile_pool(name="xpool", bufs=4) as xpool,
                tc.tile_pool(name="cspool", bufs=2) as cspool,
                tc.tile_pool(name="ropetmp", bufs=2) as ropetmp,
                tc.tile_pool(name="psA", bufs=1, space="PSUM") as psA,
                tc.tile_pool(name="psVT", bufs=2, space="PSUM") as psVT,
            ):
                wq_r = wq.rearrange("(t p) m -> p t m", p=128)
                wk_r = wk.rearrange("(t p) m -> p t m", p=128)
                wv_r = wv.rearrange("(t p) m -> p t m", p=128)
                xT_r = xT.rearrange("(t p) s -> p t s", p=128)

                # g=0 weights/x as per-ktile tiles so the very first matmul
                # only waits on ~0.5 MB of DMA
                wq0a = wpool.tile([128, 1, QS], BF16, name="wq0a")
                wq0b = wpool.tile([128, 3, QS], BF16, name="wq0b")
                xt0a = wpool.tile([128, 1, 512], BF16, name="xt0a")
                xt0b = wpool.tile([128, 3, 512], BF16, name="xt0b")
                wq_cs, wk_cs, wv_cs = [None], [], []
                for c in range(8):
                    if c > 0:
                        wq_cs.append(wpool.tile([128, 4, QS], BF16, name=f"wqc{c}"))
                    wk_cs.append(wpool.tile([128, 4, HEAD_DIM], BF16, name=f"wkc{c}"))
                    wv_cs.append(wpool.tile([128, 4, HEAD_DIM], BF16, name=f"wvc{c}"))

                pending_vt = []  # deferred V transposes (per seq block)

                def flush_vt():
                    for fn in pending_vt:
                        fn()
                    pending_vt.clear()

                for sb in range(NSB):
                    ss = slice(sb * 512, (sb + 1) * 512)
                    q_ps = [psA.tile([128, 512], F32, tag=f"qps{h}", name=f"qps{h}")
                            for h in range(QH)]
                    k_ps = psA.tile([128, 512], F32, tag="kps")
                    v_ps = psA.tile([128, 512], F32, tag="vps")

                    for g in range(8):
                        if sb == 0:
                            if g == 0:
                                # fine-grained startup: wq/x per k-tile,
                                # smallest-first so matmul 0 starts early
                                nc.sync.dma_start(out=wq0a, in_=wq_r[:, 0:1, :])
                                nc.scalar.dma_start(out=xt0a, in_=xT_r[:, 0:1, ss])
                                nc.sync.dma_start(out=wk_cs[0], in_=wk_r[:, 0:4, :])
                                nc.sync.dma_start(out=wv_cs[0], in_=wv_r[:, 0:4, :])
                                nc.sync.dma_start(out=wq0b, in_=wq_r[:, 1:4, :])
                                nc.scalar.dma_start(out=xt0b, in_=xT_r[:, 1:4, ss])
                            else:
                                nc.sync.dma_start(
                                    out=wq_cs[g], in_=wq_r[:, g * 4:(g + 1) * 4, :]
                                )
                                nc.sync.dma_start(
                                    out=wk_cs[g], in_=wk_r[:, g * 4:(g + 1) * 4, :]
                                )
                                nc.sync.dma_start(
                                    out=wv_cs[g], in_=wv_r[:, g * 4:(g + 1) * 4, :]
                                )
                            if g == 1:
                                # small constants: off the startup critical path
                                nc.sync.dma_start(out=tri_sb, in_=tri[:, :])
                                nc.sync.dma_start(out=ident_sb, in_=ident[:, :])
                                nc.sync.dma_start(
                                    out=vext[:, :, 128:129], in_=onesv[:, :, :]
                                )
                        if not (sb == 0 and g == 0):
                            xt = xpool.tile([128, 4, 512], BF16, tag="xt")
                            nc.scalar.dma_start(
                                out=xt, in_=xT_r[:, g * 4:(g + 1) * 4, ss]
                            )
                        if g == 2:
                            flush_vt()  # previous block's V transposes

                        def wq_ap(i, h):
                            if g == 0:
                                if i == 0:
                                    return wq0a[:, 0, h * 128:(h + 1) * 128]
                                return wq0b[:, i - 1, h * 128:(h + 1) * 128]
                            return wq_cs[g][:, i, h * 128:(h + 1) * 128]

                        def x_ap(i):
                            if sb == 0 and g == 0:
                                if i == 0:
                                    return xt0a[:, 0, :]
                                return xt0b[:, i - 1, :]
                            return xt[:, i, :]

                        if g < 7:
                            for i in range(4):
                                kt = g * 4 + i
                                st = (kt == 0)
                                for h in range(QH):
                                    nc.tensor.matmul(
                                        q_ps[h], wq_ap(i, h), x_ap(i),
                                        start=st, stop=False,
                                    )
                                nc.tensor.matmul(
                                    k_ps, wk_cs[g][:, i, :], x_ap(i),
                                    start=st, stop=False,
                                )
                                nc.tensor.matmul(
                                    v_ps, wv_cs[g][:, i, :], x_ap(i),
                                    start=st, stop=False,
                                )
                        else:
                            # last super-tile: head-outer so q_ps[0] stops
                            # ~20 matmuls early and its RoPE overlaps the tail
                            for h in range(QH):
                                for i in range(4):
                                    nc.tensor.matmul(
                                        q_ps[h], wq_ap(i, h), x_ap(i),
                                        start=False, stop=(i == 3),
                                    )
                            for i in range(4):
                                nc.tensor.matmul(
                                    k_ps, wk_cs[g][:, i, :], x_ap(i),
                                    start=False, stop=(i == 3),
                                )
                            for i in range(4):
                                nc.tensor.matmul(
                                    v_ps, wv_cs[g][:, i, :], x_ap(i),
                                    start=False, stop=(i == 3),
                                )

                    # RoPE tables for this block
                    cos_t = cspool.tile([128, 512], F32, tag="cos")
                    nc.sync.dma_start(out=cos_t, in_=cosT[:, ss])
                    sin_t = cspool.tile([128, 512], F32, tag="sin")
                    nc.sync.dma_start(out=sin_t, in_=sinTs[:, ss])

                    def rope(dst, src_ps):
                        # DVE multiplies the straight view directly from PSUM
                        # (partition-aligned); only the half-rotated view needs
                        # ACT cross-partition copies.  PSUM bank frees after
                        # max(ACT copies, DVE mul).
                        vr = ropetmp.tile([128, 512], F32, tag="vr", name="vr")
                        nc.scalar.copy(vr[0:64, :], src_ps[64:128, :])
                        nc.scalar.copy(vr[64:128, :], src_ps[0:64, :])
                        t = ropetmp.tile([128, 512], F32, tag="t", name="t")
                        u = ropetmp.tile([128, 512], F32, tag="u", name="u")
                        nc.vector.tensor_mul(t, src_ps, cos_t)
                        nc.vector.tensor_mul(u, vr, sin_t)
                        nc.vector.tensor_add(dst, t, u)

                    for h in range(QH):
                        rope(qTs[sb][:, h, :], q_ps[h])
                    rope(kTs[sb], k_ps)

                    # V^T -> V-natural bf16 via PE transposes; deferred into
                    # the next block's matmul stream (or phase B for sb=3)
                    vt_sb = vtbo.tile([128, 512], F32R, tag="vt")
                    nc.scalar.copy(vt_sb, v_ps)

                    def mk_vt(sb, vt_sb):
                        def emit(pool=psVT, tag="vtp"):
                            for j in range(4):
                                vt_ps = pool.tile([128, 128], F32R,
                                                  tag=tag, name=tag)
                                nc.tensor.transpose(
                                    vt_ps, vt_sb[:, j * 128:(j + 1) * 128],
                                    ident_sb,
                                )
                                nc.scalar.copy(
                                    vext[:, sb * 4 + j, 0:128], vt_ps
                                )
                        return emit

                    if sb < NSB - 1:
                        pending_vt.append(mk_vt(sb, vt_sb))
                    else:
                        # keep the PE busy through the pool-boundary barrier
                        # (rope tail) so HAM stays at full clock into phase B;
                        # transposes don't count as HAM activity, so use real
                        # matmuls
                        for i in range(24):
                            dm_ps = psA.tile([128, 128], F32, tag="qps0",
                                             name="qps0")
                            nc.tensor.matmul(
                                dm_ps, ident_sb, ident_sb,
                                start=True, stop=True,
                            )
                        mk_vt(sb, vt_sb)()
                        for i in range(8):
                            dm_ps = psA.tile([128, 128], F32, tag="qps0",
                                             name="qps0")
                            nc.tensor.matmul(
                                dm_ps, ident_sb, ident_sb,
                                start=True, stop=True,
                            )

            # ---------------- Phase B/C: attention + out projection ----------------
            with (
                tc.tile_pool(name="wopool", bufs=1) as wopool,
                tc.tile_pool(name="expp", bufs=8) as expp,
                tc.tile_pool(name="dpool", bufs=6) as dpool,
                tc.tile_pool(name="ospool", bufs=6) as ospool,
                tc.tile_pool(name="otpool", bufs=2) as otpool,
                tc.tile_pool(name="outev", bufs=4) as outev,
                tc.tile_pool(name="psS", bufs=4, space="PSUM") as psS,
                tc.tile_pool(name="psO", bufs=1, space="PSUM") as psO,
            ):
                wo_sb = wopool.tile([128, QH, DIM], F32R)
                wo_r = wo.rearrange("(h p) n -> p h n", p=128)
                for h in range(QH):
                    for c in range(2):
                        nc.sync.dma_start(
                            out=wo_sb[:, h, c * 2048:(c + 1) * 2048],
                            in_=wo_r[:, h, c * 2048:(c + 1) * 2048],
                        )

                pending_t = []  # deferred O transposes

                def flush_t():
                    for fn in pending_t:
                        fn()
                    pending_t.clear()

                for qb in range(NSB):
                    n_kb = 4 * qb + 4
                    otT_sb = otpool.tile([128, QH, 512], F32R, tag="ott",
                                         name="ott")
                    for h in range(QH):
                        o_acc = [None] * QH
                        ess = [None] * n_kb

                        def drain(kb, h=h, qb=qb, o_acc=o_acc, ess=ess,
                                  otT_sb=otT_sb):
                            if kb == 0:
                                for qc in range(QH):
                                    o_acc[qc] = psO.tile(
                                        [128, 129], F32, tag=f"oacc{qc}",
                                        name=f"oacc{qc}",
                                    )
                            for qc in range(4):
                                if 4 * qb + qc < kb:
                                    continue
                                stop = (kb == 4 * qb + qc)
                                nc.tensor.matmul(
                                    o_acc[qc],
                                    ess[kb][:, qc * 128:(qc + 1) * 128],
                                    vext[:, kb, 0:129],
                                    start=(kb == 0), stop=stop,
                                )
                                if stop:
                                    dinv = dpool.tile([128, 1], F32,
                                                      tag="dinv", name="dinv")
                                    nc.vector.reciprocal(
                                        dinv, o_acc[qc][:, 128:129]
                                    )
                                    o_sb = ospool.tile([128, 128], F32R,
                                                       tag="osb", name="osb")
                                    nc.scalar.activation(
                                        o_sb, o_acc[qc][:, 0:128],
                                        mybir.ActivationFunctionType.Copy,
                                        scale=dinv,
                                    )

                                    def mk_tp(h=h, qc=qc, o_sb=o_sb,
                                              otT_sb=otT_sb):
                                        def emit():
                                            tp = psO.tile([128, 128], F32R,
                                                          tag=f"oacc{qc}",
                                                          name=f"oacc{qc}")
                                            nc.tensor.transpose(
                                                tp, o_sb, ident_sb
                                            )
                                            nc.vector.tensor_copy(
                                                otT_sb[:, h,
                                                       qc * 128:(qc + 1) * 128],
                                                tp,
                                            )
                                        return emit

                                    pending_t.append(mk_tp())

                        for kb in range(n_kb):
                            j = kb - 4 * qb
                            lo = max(0, j) * 128
                            s_ps = psS.tile([128, 512], F32, tag="sps",
                                            name="sps")
                            nc.tensor.matmul(
                                s_ps[:, lo:512],
                                kTs[kb // 4][:, (kb % 4) * 128:
                                             (kb % 4 + 1) * 128],
                                qTs[qb][:, h, lo:512],
                                start=True, stop=True,
                            )
                            if j >= 0:
                                nc.vector.tensor_add(
                                    s_ps[:, lo:lo + 128],
                                    s_ps[:, lo:lo + 128],
                                    tri_sb,
                                )
                            es = expp.tile([128, 512], BF16, tag="es",
                                           name="es")
                            nc.scalar.activation(
                                es[:, lo:512], s_ps[:, lo:512],
                                mybir.ActivationFunctionType.Exp,
                                scale=SCALE,
                            )
                            ess[kb] = es
                            if kb == 3:
                                flush_t()  # previous head's O transposes
                            lag = 6 if h == 0 else LAG
                            if kb >= lag:
                                drain(kb - lag)
                        for kb in range(max(0, n_kb - lag), n_kb):
                            drain(kb)
                        if qb == 0:
                            # B(qb=0) is too sparse to keep HAM at full
                            # clock; pad with cheap real matmuls
                            for i in range(5):
                                dm = psS.tile([128, 128], F32, tag="sps",
                                              name="dm")
                                nc.tensor.matmul(
                                    dm, ident_sb, ident_sb,
                                    start=True, stop=True,
                                )

                    flush_t()

                    # Phase C for this query block
                    for qc in range(4):
                        for nb in range(8):
                            o_ps = psO.tile([128, 512], F32,
                                            tag=f"oacc{(qc * 8 + nb) % 4}",
                                            name="ops")
                            for h in range(QH):
                                nc.tensor.matmul(
                                    o_ps,
                                    otT_sb[:, h, qc * 128:(qc + 1) * 128],
                                    wo_sb[:, h, nb * 512:(nb + 1) * 512],
                                    start=(h == 0), stop=(h == QH - 1),
                                )
                            ob = outev.tile([128, 512], F32, tag="ob",
                                            name="ob")
                            nc.vector.tensor_copy(ob, o_ps)
                            dma_eng = nc.sync if nb % 2 == 0 else nc.gpsimd
                            dma_eng.dma_start(
                                out=out[qb * 512 + qc * 128:
                                        qb * 512 + (qc + 1) * 128,
                                        nb * 512:(nb + 1) * 512],
                                in_=ob,
                            )
    nc.finalize()
    return nc


_NC_CACHE = {}


def _get_nc():
    if "nc" not in _NC_CACHE:
        _NC_CACHE["nc"] = build_nc()
    return _NC_CACHE["nc"]


def _host_prep(x, cos, sin, mask, wq, wk, wv, wo):
    xT = np.ascontiguousarray(x[0].T.astype(ml_dtypes.bfloat16))
    cosT = np.ascontiguousarray(cos[:, 0, :].T.astype(np.float32))
    sinT = sin[:, 0, :].T.astype(np.float32)
    sinTs = np.ascontiguousarray(
        np.concatenate([-sinT[:64], sinT[64:]], axis=0)
    )
    rr = np.arange(128, dtype=np.int64)[:, None]
    cc = np.arange(128, dtype=np.int64)[None, :]
    tri = np.where(rr > cc, NEG, 0.0).astype(np.float32)
    ident = np.eye(128, dtype=np.float32)
    onesv = np.ones((128, NKB, 1), dtype=ml_dtypes.bfloat16)

    in_maps = []
    for i in range(N_CORES):
        in_maps.append({
            "xT": xT,
            "wq": np.ascontiguousarray(wq[:, i * QS:(i + 1) * QS].astype(ml_dtypes.bfloat16)),
            "wk": np.ascontiguousarray(wk[:, i * 128:(i + 1) * 128].astype(ml_dtypes.bfloat16)),
            "wv": np.ascontiguousarray(wv[:, i * 128:(i + 1) * 128].astype(ml_dtypes.bfloat16)),
            "wo": np.ascontiguousarray(wo[i * QS:(i + 1) * QS, :]),
            "cosT": cosT,
            "sinTs": sinTs,
            "tri": tri,
            "ident": ident,
            "onesv": onesv,
        })
    return in_maps


def kernel(x, cos, sin, mask, wq, wk, wv, wo, _trace=False, _trace_kwargs=None):
    nc = _get_nc()
    in_maps = _host_prep(x, cos, sin, mask, wq, wk, wv, wo)
    res = run_bass_kernel_spmd(
        nc, in_maps, list(range(N_CORES)), trace=_trace,
        **(_trace_kwargs or {}),
    )
    partials = [res.results[i]["out"] for i in range(N_CORES)]
    full = np.sum(np.stack(partials, axis=0), axis=0, dtype=np.float64)
    out = full.astype(np.float32)[None, :, :]
    if _trace:
        return out, res
    return out
